# revision 8
# baseline (speedup 1.0000x reference)
"""GCN layer kernel for Trainium2, 8 NeuronCores (SPMD).

Math (see reference):
    deg = scatter_add(ones, row); deg = max(deg, 1)
    norm_e = rsqrt(deg[row_e]) * rsqrt(deg[col_e])
    agg[row_e] += x[col_e] * norm_e
    out = agg @ W.T + b

Device strategy (v2):
  - Shard DESTINATION nodes across 8 cores (12500 each) -> no collective.
  - Edges sorted by (core, superwindow, chunk, window); windows are 256
    consecutive destination nodes, superwindows group SW=6 windows so one
    dma_gather call per (superwindow, chunk) covers ~50 tiles (994ns fixed
    SWDGE cost amortized: 36 calls/core instead of 196).
  - All data bf16: gather moves 256B/edge instead of 512B; the onehot
    tensor_scalar hits the DVE 4x_2p fast mode (all-SBUF, 2-byte packed);
    matmuls run 1 cycle/row at any width.
  - Scatter-add on the TensorEngine: aggT[f, d] += msgs[e, f]^T @ oh[e, d]
    with oh[e, d] = (iota[d] == dloc_e) * norm_e built by one fused DVE
    tensor_scalar per 128-edge tile, accumulating in PSUM per window.
  - Epilogue per window: bias is preloaded into PSUM via a rank-1 matmul
    (ones^T @ b), aggT streams through W^T on top of it, and the Act engine
    copies PSUM->SBUF, keeping the DVE free for onehots.

Host-side work is limited to index preprocessing (sort/shard/pad, int16
tables, degree/norm coefficients, dtype casts) and final unpadding/concat.
"""

import numpy as np
from contextlib import ExitStack

N_NODES = 100000
N_EDGES = 1600000
D = 128
NCORES = 8
NLOC = N_NODES // NCORES          # 12500 real dests per core
WD = 256                          # dest window
NWIN = (NLOC + WD - 1) // WD      # 49 windows (12544 padded dests)
NPAD = NWIN * WD                  # 12544
CHUNK = 32768                     # x chunk rows (int16 index range)
NCHUNK = (N_NODES + CHUNK - 1) // CHUNK  # 4
P = 128
SW = 6                            # windows per superwindow (gather batch)
NSW = (NWIN + SW - 1) // SW       # 9


def _host_prep(x, edge_index, W, b):
    """Sort/shard/pad edges; build per-core device arrays and the static
    schedule (gather calls, per-window tile lists)."""
    row = np.asarray(edge_index[0], dtype=np.int64)
    col = np.asarray(edge_index[1], dtype=np.int64)

    deg = np.bincount(row, minlength=N_NODES).astype(np.float32)
    deg = np.maximum(deg, 1.0)
    rs = 1.0 / np.sqrt(deg)
    norm = (rs[row] * rs[col]).astype(np.float32)

    core = row // NLOC
    local = row - core * NLOC
    win = local // WD
    sw = win // SW
    chunk = col >> 15
    key = (((core * NSW + sw) * NCHUNK + chunk) * NWIN + win).astype(np.int64)
    order = np.argsort(key, kind="stable")
    col_s = col[order]
    local_s = local[order]
    norm_s = norm[order]
    core_s = core[order]
    win_s = win[order]
    chunk_s = chunk[order]

    # counts per (core, win, chunk) -> shared tile table
    key2 = ((core_s * NWIN + win_s) * NCHUNK + chunk_s).astype(np.int64)
    counts = np.bincount(key2, minlength=NCORES * NWIN * NCHUNK).reshape(
        NCORES, NWIN, NCHUNK)
    T_wc = np.ceil(counts.max(axis=0) / P).astype(np.int64)  # [NWIN, NCHUNK]

    # global tile order: (sw, chunk, win-within-sw); also the msgs slot order
    tile_col = np.zeros((NWIN, NCHUNK), dtype=np.int64)  # global tile idx
    gather_calls = []   # (sw, chunk, gt0, ntiles)
    sw_start = []       # first global tile of each sw
    gt = 0
    for s in range(NSW):
        ws = range(s * SW, min((s + 1) * SW, NWIN))
        sw_start.append(gt)
        for c in range(NCHUNK):
            t0 = gt
            for w in ws:
                tile_col[w, c] = gt
                gt += int(T_wc[w, c])
            if gt > t0:
                gather_calls.append((s, c, t0, gt - t0))
    total_tiles = gt
    sw_start.append(gt)
    total_edges_padded = total_tiles * P
    max_sw_tiles = max(sw_start[s + 1] - sw_start[s] for s in range(NSW))

    # sorted-edge start offsets per (core, sw, chunk, win) group == key order
    key_s = key[order]
    ngroups = NCORES * NSW * NCHUNK * NWIN
    gcounts = np.bincount(key_s, minlength=ngroups)
    gstarts = np.zeros(ngroups + 1, dtype=np.int64)
    np.cumsum(gcounts, out=gstarts[1:])

    import concourse.mybir as mybir
    bf16 = mybir.dt.np(mybir.dt.bfloat16)

    x16 = np.asarray(x, np.float32).astype(bf16)
    wt16 = np.ascontiguousarray(np.asarray(W, np.float32).T).astype(bf16)
    b16 = np.asarray(b, np.float32).reshape(1, D).astype(bf16)
    ones16 = np.ones((1, D), np.float32).astype(bf16)
    iota16 = np.tile(np.arange(WD, dtype=np.float32), (P, 1)).astype(bf16)

    idx_cols = total_edges_padded // 16
    in_maps = []
    for k in range(NCORES):
        dl_flat = np.zeros(total_edges_padded, np.float32)
        nm_flat = np.zeros(total_edges_padded, np.float32)
        ix_flat = np.zeros(total_edges_padded, np.int16)
        for s in range(NSW):
            ws = range(s * SW, min((s + 1) * SW, NWIN))
            for c in range(NCHUNK):
                for w in ws:
                    g = ((k * NSW + s) * NCHUNK + c) * NWIN + w
                    st, e = gstarts[g], gstarts[g + 1]
                    n = int(e - st)
                    if n == 0:
                        continue
                    off = int(tile_col[w, c]) * P
                    sl = slice(off, off + n)
                    ix_flat[sl] = (col_s[st:e] - (c << 15)).astype(np.int16)
                    dl_flat[sl] = (local_s[st:e] - w * WD).astype(np.float32)
                    nm_flat[sl] = norm_s[st:e]
        # gathered row i of a call lands at msgs[p=i%128, tile i//128]:
        # per-tile column layout for dloc/nrm = [P, total_tiles]
        dloc2 = dl_flat.reshape(total_tiles, P).T
        nrm2 = nm_flat.reshape(total_tiles, P).T
        consts = np.concatenate([dloc2, nrm2], axis=1).astype(np.float32)
        # idx table: wrapped in 16 partitions (i -> [i%16, i//16]),
        # replicated to 128 partitions (8 q7 cores)
        idx16 = np.ascontiguousarray(ix_flat.reshape(idx_cols, 16).T)
        idx128 = np.tile(idx16, (8, 1))
        in_maps.append({
            "x": x16,
            "idxs": idx128,
            "consts": consts,
            "wt": wt16,
            "bvec": b16,
            "ones": ones16,
            "iota": iota16,
        })

    plan = dict(
        T_wc=T_wc, tile_col=tile_col, gather_calls=gather_calls,
        sw_start=sw_start, total_tiles=total_tiles, idx_cols=idx_cols,
        max_sw_tiles=max_sw_tiles,
    )
    return in_maps, plan


def _build_nc(plan, repeat=1, parts=("gather", "onehot", "matmul", "epilogue"),
              nqueues=4, sp=False, gbufs=2, ohbufs=6, dyn_repeat=False):
    from concourse import bacc, mybir
    import concourse.tile as tile

    f32 = mybir.dt.float32
    bf16 = mybir.dt.bfloat16
    i16 = mybir.dt.int16

    T_wc = plan["T_wc"]
    tile_col = plan["tile_col"]
    gather_calls = plan["gather_calls"]
    sw_start = plan["sw_start"]
    total_tiles = plan["total_tiles"]
    idx_cols = plan["idx_cols"]
    max_sw_tiles = plan["max_sw_tiles"]

    nc = bacc.Bacc("TRN2", num_swdge_queues=nqueues)
    x_ext = nc.declare_dram_parameter("x", [N_NODES, D], bf16, isOutput=False)
    idx_ext = nc.declare_dram_parameter("idxs", [P, idx_cols], i16, isOutput=False)
    c_ext = nc.declare_dram_parameter("consts", [P, 2 * total_tiles], f32, isOutput=False)
    wt_ext = nc.declare_dram_parameter("wt", [D, D], bf16, isOutput=False)
    b_ext = nc.declare_dram_parameter("bvec", [1, D], bf16, isOutput=False)
    ones_ext = nc.declare_dram_parameter("ones", [1, D], bf16, isOutput=False)
    iota_ext = nc.declare_dram_parameter("iota", [P, WD], bf16, isOutput=False)
    if dyn_repeat:
        nrep_ext = nc.declare_dram_parameter("nrep", [1, 1], mybir.dt.int32, isOutput=False)
    out_ext = nc.declare_dram_parameter("out", [NPAD, D], f32, isOutput=True)

    with tile.TileContext(nc) as tc:
        with ExitStack() as ctx:
            const = ctx.enter_context(tc.tile_pool(name="const", bufs=1))
            gat = ctx.enter_context(tc.tile_pool(name="gat", bufs=gbufs))
            oh_pool = ctx.enter_context(tc.tile_pool(name="oh", bufs=ohbufs))
            ep = ctx.enter_context(tc.tile_pool(name="ep", bufs=3))
            psum = ctx.enter_context(tc.tile_pool(name="psum", bufs=2, space="PSUM"))
            psum_o = ctx.enter_context(tc.tile_pool(name="psum_o", bufs=2, space="PSUM"))

            idx_sb = const.tile([P, idx_cols], i16)
            nc.sync.dma_start(idx_sb[:], idx_ext[:])
            c_sb = const.tile([P, 2 * total_tiles], f32)
            nc.sync.dma_start(c_sb[:], c_ext[:])
            wt_sb = const.tile([D, D], bf16)
            nc.sync.dma_start(wt_sb[:], wt_ext[:])
            b_sb = const.tile([1, D], bf16)
            nc.sync.dma_start(b_sb[:], b_ext[:])
            ones_sb = const.tile([1, D], bf16)
            nc.sync.dma_start(ones_sb[:], ones_ext[:])
            iota_sb = const.tile([P, WD], bf16)
            nc.sync.dma_start(iota_sb[:], iota_ext[:])

            fake_msgs = None
            if "fakegather" in parts:
                fake_msgs = const.tile([P, max_sw_tiles * D], bf16)
                nc.sync.dma_start(
                    fake_msgs[:],
                    x_ext[0:P * max_sw_tiles, :].rearrange("(p t) d -> p (t d)", p=P))

            rep_ctx = None
            if dyn_repeat:
                nrep_sb = const.tile([1, 1], mybir.dt.int32)
                nc.sync.dma_start(nrep_sb[:], nrep_ext[:])
                nrep_val = nc.values_load(nrep_sb[:], min_val=0, max_val=1 << 20)
                rep_ctx = tc.For_i(0, nrep_val, 1)
                rep_ctx.__enter__()

            qn = 0
            for _rep in range(repeat):
                for s in range(NSW):
                    ws = range(s * SW, min((s + 1) * SW, NWIN))
                    if fake_msgs is not None:
                        msgs = fake_msgs
                    else:
                        msgs = gat.tile([P, max_sw_tiles * D], bf16, tag="msgs")
                    if "gather" in parts and fake_msgs is None:
                        for (s_, c, gt0, ntiles) in gather_calls:
                            if s_ != s:
                                continue
                            col0 = gt0 - sw_start[s]
                            nc.gpsimd.dma_gather(
                                out_ap=msgs[:, col0 * D:(col0 + ntiles) * D]
                                .rearrange("p (c d) -> p c d", d=D),
                                in_ap=x_ext[c * CHUNK:min((c + 1) * CHUNK, N_NODES), :],
                                idxs_ap=idx_sb[:, gt0 * (P // 16):(gt0 + ntiles) * (P // 16)],
                                num_idxs=ntiles * P,
                                num_idxs_reg=ntiles * P,
                                elem_size=D,
                                single_packet=sp,
                                queue_num=qn % nqueues,
                            )
                            qn += 1
                    for w in ws:
                        tlist = []
                        for c in range(NCHUNK):
                            g0 = int(tile_col[w, c])
                            for t in range(int(T_wc[w, c])):
                                tlist.append((g0 + t - sw_start[s], g0 + t))
                        if not tlist:
                            continue
                        aggT_ps = psum.tile([P, WD], f32, space="PSUM")
                        for i, (colsw, gidx) in enumerate(tlist):
                            oh = oh_pool.tile([P, WD], bf16)
                            if "onehot" in parts:
                                nc.vector.tensor_scalar(
                                    out=oh[:],
                                    in0=iota_sb[:],
                                    scalar1=c_sb[:, gidx:gidx + 1],
                                    scalar2=c_sb[:, total_tiles + gidx:total_tiles + gidx + 1],
                                    op0=mybir.AluOpType.is_equal,
                                    op1=mybir.AluOpType.mult,
                                )
                            if "matmul" in parts:
                                nc.tensor.matmul(
                                    out=aggT_ps[:],
                                    lhsT=msgs[:, colsw * D:(colsw + 1) * D],
                                    rhs=oh[:],
                                    start=(i == 0),
                                    stop=(i == len(tlist) - 1),
                                )
                        if "epilogue" not in parts or "matmul" not in parts:
                            continue
                        aggT_sb = ep.tile([P, WD], bf16, tag="aggT")
                        nc.scalar.copy(aggT_sb[:], aggT_ps[:])
                        for h in range(WD // P):
                            out_ps = psum_o.tile([P, D], f32, space="PSUM")
                            nc.tensor.matmul(
                                out=out_ps[:],
                                lhsT=ones_sb[:],
                                rhs=b_sb[:],
                                start=True, stop=False,
                            )
                            nc.tensor.matmul(
                                out=out_ps[:],
                                lhsT=aggT_sb[:, h * P:(h + 1) * P],
                                rhs=wt_sb[:],
                                start=False, stop=True,
                            )
                            out_sb = ep.tile([P, D], f32, tag="out")
                            nc.scalar.copy(out_sb[:], out_ps[:])
                            nc.sync.dma_start(
                                out_ext[w * WD + h * P: w * WD + (h + 1) * P, :],
                                out_sb[:],
                            )

            if rep_ctx is not None:
                rep_ctx.__exit__(None, None, None)

    nc.compile()
    return nc


def run(x, edge_index, W, b, trace=False, **build_kwargs):
    """Build + run on 8 cores. Returns (out, results)."""
    from concourse.bass_utils import run_bass_kernel_spmd

    in_maps, plan = _host_prep(x, edge_index, W, b)
    nc = _build_nc(plan, **build_kwargs)
    res = run_bass_kernel_spmd(nc, in_maps, list(range(NCORES)), trace=trace)
    parts = [res.results[k]["out"][:NLOC] for k in range(NCORES)]
    out = np.concatenate(parts, axis=0).astype(np.float32)
    return out, res


def kernel(x, edge_index, W, b):
    out, _ = run(x, edge_index, W, b)
    return out


# ---------------------------------------------------------------------------
# benchmarking: time repeat=R vs repeat=1 NEFFs with device-resident inputs;
# the delta cancels transfers/dispatch and yields per-iteration HW time.
# ---------------------------------------------------------------------------

def _make_callable(nc, in_maps):
    import jax
    import numpy as _np
    from jax.sharding import Mesh, PartitionSpec, NamedSharding
    from jax.experimental.shard_map import shard_map
    from concourse import mybir
    from concourse.bass2jax import (
        _bass_exec_p, install_neuronx_cc_hook, partition_id_tensor,
    )

    install_neuronx_cc_hook()
    n_cores = len(in_maps)
    in_names, out_names, out_avals, zero_outs = [], [], [], []
    for alloc in nc.m.functions[0].allocations:
        if not isinstance(alloc, mybir.MemoryLocationSet):
            continue
        name = alloc.memorylocations[0].name
        if alloc.kind == "ExternalInput":
            if nc.partition_id_tensor is None or name != nc.partition_id_tensor.name:
                in_names.append(name)
        elif alloc.kind == "ExternalOutput":
            out_names.append(name)
            shape = tuple(alloc.tensor_shape)
            dtype = mybir.dt.np(alloc.dtype)
            out_avals.append(jax.core.ShapedArray(shape, dtype))
            zero_outs.append(_np.zeros(shape, dtype))
    n_params = len(in_names)
    all_in_names = in_names + out_names
    if nc.partition_id_tensor is not None:
        all_in_names = all_in_names + [nc.partition_id_tensor.name]

    def _body(*args):
        operands = list(args)
        if nc.partition_id_tensor is not None:
            operands.append(partition_id_tensor())
        outs = _bass_exec_p.bind(
            *operands,
            out_avals=tuple(out_avals),
            in_names=tuple(all_in_names),
            out_names=tuple(out_names),
            lowering_input_output_aliases=(),
            sim_require_finite=True,
            sim_require_nnan=True,
            nc=nc,
        )
        return tuple(outs)

    devices = jax.devices()[:n_cores]
    mesh = Mesh(_np.asarray(devices), ("core",))
    spec = PartitionSpec("core")
    in_specs = (spec,) * (n_params + len(out_names))
    out_specs = (spec,) * len(out_names)
    fn = jax.jit(shard_map(_body, mesh=mesh, in_specs=in_specs,
                           out_specs=out_specs, check_rep=False),
                 keep_unused=True)
    sharding = NamedSharding(mesh, spec)
    dev_in = [
        jax.device_put(
            _np.concatenate([_np.asarray(in_maps[c][nm]) for c in range(n_cores)], axis=0),
            sharding)
        for nm in in_names
    ]
    dev_zero = [
        jax.device_put(_np.zeros((n_cores * z.shape[0], *z.shape[1:]), z.dtype), sharding)
        for z in zero_outs
    ]
    return fn, dev_in, dev_zero, in_names, sharding


def bench_nc(in_maps, plan, K=9, iters=20, **build_kwargs):
    """HW time per iteration via unrolled-repeat NEFFs (repeat=1 vs K),
    timed interleaved so tunnel-dispatch drift cancels. Returns
    (per_iter_ns, info)."""
    import time
    import jax

    nc1 = _build_nc(plan, repeat=1, **build_kwargs)
    fn1, di1, dz1, _, _ = _make_callable(nc1, in_maps)
    ncK = _build_nc(plan, repeat=K, **build_kwargs)
    fnK, diK, dzK, _, _ = _make_callable(ncK, in_maps)

    outs = fn1(*di1, *dz1)
    jax.block_until_ready(outs)
    out1 = np.asarray(outs[0])
    jax.block_until_ready(fnK(*diK, *dzK))

    w1 = wK = float("inf")
    h1, hK = [], []
    for _ in range(iters):
        t0 = time.perf_counter()
        jax.block_until_ready(fn1(*di1, *dz1))
        dt = time.perf_counter() - t0
        h1.append(dt)
        w1 = min(w1, dt)
        t0 = time.perf_counter()
        jax.block_until_ready(fnK(*diK, *dzK))
        dt = time.perf_counter() - t0
        hK.append(dt)
        wK = min(wK, dt)
    per_iter_ns = (wK - w1) / (K - 1) * 1e9
    return per_iter_ns, {"w1": w1, "wK": wK, "K": K, "out1": out1,
                         "h1": h1, "hK": hK}
